# revision 24
# baseline (speedup 1.0000x reference)
"""NF4-quantized LoRA linear layer on 8 Trainium2 NeuronCores.

Computation (reference):
    w = NF4_TABLE[w_codes] * w_scales[block-expanded]        # [O, I]
    out = x @ w.T + (alpha/rank) * (x @ lora_a.T) @ lora_b.T # [B, S, O]

Strategy (v2):
  - Tensor-parallel split of the output dim across 8 cores (O_SH = 512 each).
    Every core sees all of x; no collectives; host concatenates outputs.
  - Host re-encodes the 4-bit codes as their f16 NF4 table values (a
    bijective per-element recode, same spirit as the baseline's f16 cast of
    the integer codes); the device still applies the per-block scales and
    folds the LoRA product into the weights:
        W_eff = t * s + (alpha/rank) * lora_a.T @ lora_b.T
    W assembly is 2 DVE passes per tile instead of a 31-pass spline chain,
    so the m-loop is a single phase with no DRAM partial round-trip.
  - Head pipeline: PE warm-up matmuls, then per i-tile {LoRA-fold MM ->
    ACT copy -> DVE assemble} interleaved with an it-major group of the
    first 4 m-tiles so the PE is saturated while W streams in.
  - Steady state: 60 m-tiles x 32 accumulating MMs [128i x 128m @ 128i x
    512o], ACT drain, direct DMA to out.
"""

import numpy as np
import ml_dtypes

import concourse.mybir as mybir
import concourse.tile as tile
from concourse import bacc
from concourse.bass_utils import run_bass_kernel_spmd

NF4_TABLE = np.array(
    [
        -1.0, -0.6961928009986877, -0.5250730514526367, -0.39491748809814453,
        -0.28444138169288635, -0.18477343022823334, -0.09105003625154495, 0.0,
        0.07958029955625534, 0.16093020141124725, 0.24611230194568634,
        0.33791524171829224, 0.44070982933044434, 0.5626170039176941,
        0.7229568362236328, 1.0,
    ],
    dtype=np.float64,
)

B, S, I, O, R, BLK = 4, 2048, 4096, 4096, 16, 64
M = B * S                      # 8192 token rows
N_CORES = 8
O_SH = O // N_CORES            # 512 output cols per core
IT = I // 128                  # 32 contraction tiles
MT = M // 128                  # 64 row tiles
MACRO = 4                      # i-tiles per staging macro
G = 5                          # m-tiles in the it-major head group
LAG = 4                        # i-tiles of lead the W pipeline keeps
LORA_SCALE = 2.0               # alpha / rank

F16 = mybir.dt.float16
BF16 = mybir.dt.bfloat16
F32 = mybir.dt.float32
ALU = mybir.AluOpType
ACTF = mybir.ActivationFunctionType

BF16_NP = ml_dtypes.bfloat16


def _build_nc():
    nc = bacc.Bacc("TRN2", target_bir_lowering=False, debug=False,
                   num_devices=N_CORES)

    xt = nc.dram_tensor("xt", [128, MT, IT, 128], BF16, kind="ExternalInput")
    tvals = nc.dram_tensor("tvals", [I, O_SH], F16, kind="ExternalInput")
    scales = nc.dram_tensor("scales", [I, O_SH], F16, kind="ExternalInput")
    la = nc.dram_tensor("la", [R, I], BF16, kind="ExternalInput")
    lb = nc.dram_tensor("lb", [R, O_SH], BF16, kind="ExternalInput")
    out = nc.dram_tensor("out", [M, O_SH], BF16, kind="ExternalOutput")

    tvals_r = tvals.ap().rearrange("(t p) o -> p t o", p=128)
    scales_r = scales.ap().rearrange("(t p) o -> p t o", p=128)

    with tile.TileContext(nc) as tc:
        with (
            tc.tile_pool(name="wpool", bufs=IT // MACRO) as wpool,
            tc.tile_pool(name="wlpool", bufs=2) as wlpool,
            tc.tile_pool(name="dq", bufs=3) as dq,
            tc.tile_pool(name="tmppool", bufs=2) as tmppool,
            tc.tile_pool(name="xgpool", bufs=1) as xgpool,
            tc.tile_pool(name="xpool", bufs=4) as xpool,
            tc.tile_pool(name="cpool", bufs=1) as cpool,
            tc.tile_pool(name="opool", bufs=3) as opool,
            tc.tile_pool(name="ps_o", bufs=G, space="PSUM") as pp_o,
            tc.tile_pool(name="ps_l", bufs=3, space="PSUM") as pp_l,
        ):
            # constants
            la_sb = cpool.tile([R, I], BF16, tag="la")
            nc.sync.dma_start(la_sb[:], la.ap())
            lb_sb = cpool.tile([R, O_SH], BF16, tag="lb")
            nc.sync.dma_start(lb_sb[:], lb.ap())
            warm = cpool.tile([128, O_SH], BF16, tag="warm")
            nc.vector.memset(warm[:], 0.125)

            # head-group x tiles, staged in 4 sub-chunks of 8 i-tiles so the
            # it-major matmuls below can start before the full tile arrives
            xg = []
            for g in range(G):
                xa = xgpool.tile([128, IT, 128], BF16, tag=f"xg{g}",
                                 name=f"xg{g}")
                xg.append(xa)

            def stage_x_chunk(c):
                for g in range(G):
                    nc.sync.dma_start(
                        xg[g][:, c * 4:(c + 1) * 4, :],
                        xt.ap()[:, g, c * 4:(c + 1) * 4, :],
                    )

            po_g = [
                pp_o.tile([128, O_SH], F32, tag="po", name=f"pog{g}")
                for g in range(G)
            ]

            # ---- W pipeline interleaved with the head group ----
            w_aps = {}
            xa_pre = {}
            tm = sm = wlm = wtm = None

            def stage_ts_macro(k):
                fd = MACRO * O_SH
                tm_ = dq.tile([128, fd], F16, tag="tm", name=f"tm{k}")
                nc.sync.dma_start(
                    tm_[:].rearrange("p (t o) -> p t o", t=MACRO),
                    tvals_r[:, k:k + MACRO, :],
                )
                sm_ = dq.tile([128, fd], F16, tag="sm", name=f"sm{k}")
                nc.sync.dma_start(
                    sm_[:].rearrange("p (t o) -> p t o", t=MACRO),
                    scales_r[:, k:k + MACRO, :],
                )
                wlm_ = wlpool.tile([128, fd], F16, tag="wl", name=f"wl{k}")
                wtm_ = wpool.tile([128, fd], BF16, tag="w", name=f"w{k}")
                return tm_, sm_, wlm_, wtm_

            def lora_fold(k, wlm_):
                sl = slice((k % MACRO) * O_SH, (k % MACRO) * O_SH + O_SH)
                pl = pp_l.tile([128, O_SH], F32, tag="pl", name=f"pl{k}")
                nc.tensor.matmul(
                    pl[:], la_sb[:, k * 128:(k + 1) * 128], lb_sb[:],
                    start=True, stop=True,
                )
                nc.scalar.copy(wlm_[:, sl], pl[:])

            # Stage the first W macros and the head-group x before anything
            # else competes for DMA bandwidth.
            ts_staged = {}
            ts_staged[0] = stage_ts_macro(0)
            stage_x_chunk(0)
            stage_x_chunk(1)
            ts_staged[1] = stage_ts_macro(MACRO)

            # PE warm-up: keep the HAM activity window busy while the first
            # x / t / s DMAs stream in, so real matmuls start at 2.4 GHz.
            def dummies(lo, n):
                for d in range(lo, lo + n):
                    pd = pp_l.tile([128, O_SH], F32, tag="pl", name=f"pd{d}")
                    nc.tensor.matmul(pd[:], warm[:, :128], warm[:],
                                     start=True, stop=True)

            def assemble_it(k, tm_, sm_, wlm_, wtm_):
                # wt = t*s + wl for one i-tile (2 DVE passes, f16 -> bf16)
                sl = slice((k % MACRO) * O_SH, (k % MACRO) * O_SH + O_SH)
                tsm_ = tmppool.tile([128, O_SH], F16, tag="ts", name=f"ts{k}")
                nc.vector.tensor_tensor(tsm_[:], tm_[:, sl], sm_[:, sl],
                                        op=ALU.mult)
                nc.vector.tensor_tensor(wtm_[:, sl], tsm_[:], wlm_[:, sl],
                                        op=ALU.add)
                w_aps[k] = wtm_[:, sl]

            dummies(0, 6)

            # macro 0: LoRA fold + per-i-tile assembly ahead of the warm-up
            # so wt[0..3] is ready the moment the head group can issue.
            tm, sm, wlm, wtm = ts_staged[0]
            for k in range(MACRO):
                lora_fold(k, wlm)
                assemble_it(k, tm, sm, wlm, wtm)

            dummies(6, 8)

            def group_mms(jlo):
                for jt in range(jlo, jlo + MACRO):
                    for g in range(G):
                        nc.tensor.matmul(
                            po_g[g][:], xg[g][:, jt, :], w_aps[jt],
                            start=(jt == 0), stop=(jt == IT - 1),
                        )

            # macro-stepped head: macro m's LoRA fold + assembly is emitted
            # ahead of macro (m-1)'s 20 head-group matmuls, and its t/s DMA
            # one full macro-step earlier, so each ~5us group window hides
            # the next macro's W pipeline with margin.
            n_macros = IT // MACRO
            for m in range(1, n_macros + 1):
                if m < n_macros:
                    k0 = m * MACRO
                    if m + 1 < n_macros:
                        stage_x_chunk(m + 1)
                        ts_staged[m + 1] = stage_ts_macro(k0 + MACRO)
                    tm, sm, wlm, wtm = ts_staged[m]
                    for kk in range(k0, k0 + MACRO):
                        lora_fold(kk, wlm)
                    for kk in range(k0, k0 + MACRO):
                        assemble_it(kk, tm, sm, wlm, wtm)
                    if m >= 6:
                        # prefetch the first steady-state x tiles so the
                        # m-loop starts without a DMA bubble
                        for mt_pre in (G + 2 * (m - 6), G + 2 * (m - 6) + 1):
                            xa_p = xpool.tile([128, IT, 128], BF16, tag="xa",
                                              name=f"xa_pre{mt_pre}")
                            nc.sync.dma_start(xa_p[:],
                                              xt.ap()[:, mt_pre, :, :])
                            xa_pre[mt_pre] = xa_p
                group_mms((m - 1) * MACRO)

            def drain(po, mt):
                ev = opool.tile([128, O_SH], BF16, tag="ev")
                nc.scalar.copy(ev[:], po[:])
                nc.sync.dma_start(out.ap()[mt * 128:(mt + 1) * 128, :], ev[:])

            for g in range(G):
                drain(po_g[g], g)

            # ---- steady-state m-loop ----
            for mt in range(G, MT):
                if mt in xa_pre:
                    xa = xa_pre[mt]
                else:
                    xa = xpool.tile([128, IT, 128], BF16, tag="xa")
                    nc.sync.dma_start(xa[:], xt.ap()[:, mt, :, :])
                po = pp_o.tile([128, O_SH], F32, tag="po")
                for it in range(IT):
                    nc.tensor.matmul(
                        po[:], xa[:, it, :], w_aps[it],
                        start=(it == 0), stop=(it == IT - 1),
                    )
                drain(po, mt)

    nc.compile()
    return nc


_NC_CACHE = {}


def _get_nc():
    if "nc" not in _NC_CACHE:
        _NC_CACHE["nc"] = _build_nc()
    return _NC_CACHE["nc"]


def prepare_in_maps(x, w_codes, w_scales, lora_a, lora_b):
    """Host-side sharding + layout prep (casts/folds/recodes only)."""
    xm = np.ascontiguousarray(x.reshape(M, I))
    # xt[p, mt, t, mm] = x[mt*128+mm, t*128+p], bf16
    xtl = (
        xm.T.reshape(IT, 128, MT, 128)
        .transpose(1, 2, 0, 3)
        .astype(BF16_NP)
    )
    xtl = np.ascontiguousarray(xtl)

    la = np.ascontiguousarray(
        (LORA_SCALE * lora_a.astype(np.float64)).astype(BF16_NP)
    )

    tvals_full = NF4_TABLE[w_codes].astype(np.float16)          # [O, I]
    scales_full = np.repeat(
        w_scales.astype(np.float16), BLK, axis=1
    )                                                           # [O, I]

    in_maps = []
    for c in range(N_CORES):
        o_lo, o_hi = c * O_SH, (c + 1) * O_SH
        tvals_t = np.ascontiguousarray(tvals_full[o_lo:o_hi].T)
        scales_t = np.ascontiguousarray(scales_full[o_lo:o_hi].T)
        lb_t = np.ascontiguousarray(lora_b[o_lo:o_hi].T.astype(BF16_NP))
        in_maps.append(
            {
                "xt": xtl,
                "tvals": tvals_t,
                "scales": scales_t,
                "la": la,
                "lb": lb_t,
            }
        )
    return in_maps


def run(in_maps, trace=False, retries=2):
    nc = _get_nc()
    last = None
    for attempt in range(retries + 1):
        try:
            return run_bass_kernel_spmd(
                nc, in_maps, core_ids=list(range(N_CORES)), trace=trace
            )
        except Exception as e:  # transient NRT/axon device errors
            last = e
            if attempt == retries:
                raise
            import time as _time

            _time.sleep(5)
    raise last


def kernel(x, w_codes, w_scales, lora_a, lora_b):
    in_maps = prepare_in_maps(x, w_codes, w_scales, lora_a, lora_b)
    res = run(in_maps, trace=False)
    out = np.concatenate(
        [np.asarray(res.results[c]["out"]) for c in range(N_CORES)], axis=1
    )
    return out.reshape(B, S, O).astype(np.float32)


# revision 27
# speedup vs baseline: 1.0059x; 1.0059x over previous
"""NF4-quantized LoRA linear layer on 8 Trainium2 NeuronCores.

Computation (reference):
    w = NF4_TABLE[w_codes] * w_scales[block-expanded]        # [O, I]
    out = x @ w.T + (alpha/rank) * (x @ lora_a.T) @ lora_b.T # [B, S, O]

Strategy (v2):
  - Tensor-parallel split of the output dim across 8 cores (O_SH = 512 each).
    Every core sees all of x; no collectives; host concatenates outputs.
  - Host re-encodes the 4-bit codes as their f16 NF4 table values (a
    bijective per-element recode, same spirit as the baseline's f16 cast of
    the integer codes); the device still applies the per-block scales and
    folds the LoRA product into the weights:
        W_eff = t * s + (alpha/rank) * lora_a.T @ lora_b.T
    W assembly is 2 DVE passes per tile instead of a 31-pass spline chain,
    so the m-loop is a single phase with no DRAM partial round-trip.
  - Head pipeline: PE warm-up matmuls, then per i-tile {LoRA-fold MM ->
    ACT copy -> DVE assemble} interleaved with an it-major group of the
    first 4 m-tiles so the PE is saturated while W streams in.
  - Steady state: 60 m-tiles x 32 accumulating MMs [128i x 128m @ 128i x
    512o], ACT drain, direct DMA to out.
"""

import numpy as np
import ml_dtypes

import concourse.mybir as mybir
import concourse.tile as tile
from concourse import bacc
from concourse.bass_utils import run_bass_kernel_spmd

NF4_TABLE = np.array(
    [
        -1.0, -0.6961928009986877, -0.5250730514526367, -0.39491748809814453,
        -0.28444138169288635, -0.18477343022823334, -0.09105003625154495, 0.0,
        0.07958029955625534, 0.16093020141124725, 0.24611230194568634,
        0.33791524171829224, 0.44070982933044434, 0.5626170039176941,
        0.7229568362236328, 1.0,
    ],
    dtype=np.float64,
)

B, S, I, O, R, BLK = 4, 2048, 4096, 4096, 16, 64
M = B * S                      # 8192 token rows
N_CORES = 8
O_SH = O // N_CORES            # 512 output cols per core
IT = I // 128                  # 32 contraction tiles
MT = M // 128                  # 64 row tiles
MACRO = 4                      # i-tiles per staging macro
G = 5                          # m-tiles in the it-major head group
LAG = 4                        # i-tiles of lead the W pipeline keeps
LORA_SCALE = 2.0               # alpha / rank

F16 = mybir.dt.float16
BF16 = mybir.dt.bfloat16
F32 = mybir.dt.float32
ALU = mybir.AluOpType
ACTF = mybir.ActivationFunctionType

BF16_NP = ml_dtypes.bfloat16


def _build_nc():
    nc = bacc.Bacc("TRN2", target_bir_lowering=False, debug=False,
                   num_devices=N_CORES)

    xt = nc.dram_tensor("xt", [128, MT, IT, 128], BF16, kind="ExternalInput")
    tvals = nc.dram_tensor("tvals", [I, O_SH], F16, kind="ExternalInput")
    scales = nc.dram_tensor("scales", [I, O_SH], F16, kind="ExternalInput")
    la = nc.dram_tensor("la", [R, I], BF16, kind="ExternalInput")
    lb = nc.dram_tensor("lb", [R, O_SH], BF16, kind="ExternalInput")
    out = nc.dram_tensor("out", [M, O_SH], BF16, kind="ExternalOutput")

    tvals_r = tvals.ap().rearrange("(t p) o -> p t o", p=128)
    scales_r = scales.ap().rearrange("(t p) o -> p t o", p=128)

    with tile.TileContext(nc) as tc:
        with (
            tc.tile_pool(name="wpool", bufs=IT // MACRO) as wpool,
            tc.tile_pool(name="wlpool", bufs=2) as wlpool,
            tc.tile_pool(name="dq", bufs=3) as dq,
            tc.tile_pool(name="tmppool", bufs=2) as tmppool,
            tc.tile_pool(name="xgpool", bufs=1) as xgpool,
            tc.tile_pool(name="xpool", bufs=4) as xpool,
            tc.tile_pool(name="cpool", bufs=1) as cpool,
            tc.tile_pool(name="opool", bufs=3) as opool,
            tc.tile_pool(name="ps_o", bufs=G, space="PSUM") as pp_o,
            tc.tile_pool(name="ps_l", bufs=3, space="PSUM") as pp_l,
        ):
            # constants
            la_sb = cpool.tile([R, I], BF16, tag="la")
            nc.sync.dma_start(la_sb[:], la.ap())
            lb_sb = cpool.tile([R, O_SH], BF16, tag="lb")
            nc.sync.dma_start(lb_sb[:], lb.ap())
            warm = cpool.tile([128, O_SH], BF16, tag="warm")
            nc.vector.memset(warm[:], 0.125)

            # head-group x tiles, staged in 4 sub-chunks of 8 i-tiles so the
            # it-major matmuls below can start before the full tile arrives
            xg = []
            for g in range(G):
                xa = xgpool.tile([128, IT, 128], BF16, tag=f"xg{g}",
                                 name=f"xg{g}")
                xg.append(xa)

            def stage_x_chunk(c):
                for g in range(G):
                    nc.sync.dma_start(
                        xg[g][:, c * 8:(c + 1) * 8, :],
                        xt.ap()[:, g, c * 8:(c + 1) * 8, :],
                    )

            po_g = [
                pp_o.tile([128, O_SH], F32, tag="po", name=f"pog{g}")
                for g in range(G)
            ]

            # ---- W pipeline interleaved with the head group ----
            w_aps = {}
            xa_pre = {}
            tm = sm = wlm = wtm = None

            def stage_ts_macro(k):
                fd = MACRO * O_SH
                tm_ = dq.tile([128, fd], F16, tag="tm", name=f"tm{k}")
                nc.sync.dma_start(
                    tm_[:].rearrange("p (t o) -> p t o", t=MACRO),
                    tvals_r[:, k:k + MACRO, :],
                )
                sm_ = dq.tile([128, fd], F16, tag="sm", name=f"sm{k}")
                nc.sync.dma_start(
                    sm_[:].rearrange("p (t o) -> p t o", t=MACRO),
                    scales_r[:, k:k + MACRO, :],
                )
                wlm_ = wlpool.tile([128, fd], F16, tag="wl", name=f"wl{k}")
                wtm_ = wpool.tile([128, fd], BF16, tag="w", name=f"w{k}")
                return tm_, sm_, wlm_, wtm_

            def lora_fold(k, wlm_):
                sl = slice((k % MACRO) * O_SH, (k % MACRO) * O_SH + O_SH)
                pl = pp_l.tile([128, O_SH], F32, tag="pl", name=f"pl{k}")
                nc.tensor.matmul(
                    pl[:], la_sb[:, k * 128:(k + 1) * 128], lb_sb[:],
                    start=True, stop=True,
                )
                nc.scalar.copy(wlm_[:, sl], pl[:])

            # Stage the first W macros and the head-group x before anything
            # else competes for DMA bandwidth.
            ts_staged = {}
            ts_staged[0] = stage_ts_macro(0)
            stage_x_chunk(0)
            ts_staged[1] = stage_ts_macro(MACRO)

            # PE warm-up: keep the HAM activity window busy while the first
            # x / t / s DMAs stream in, so real matmuls start at 2.4 GHz.
            def dummies(lo, n):
                for d in range(lo, lo + n):
                    pd = pp_l.tile([128, O_SH], F32, tag="pl", name=f"pd{d}")
                    nc.tensor.matmul(pd[:], warm[:, :128], warm[:],
                                     start=True, stop=True)

            def assemble_it(k, tm_, sm_, wlm_, wtm_):
                # wt = t*s + wl for one i-tile (2 DVE passes, f16 -> bf16)
                sl = slice((k % MACRO) * O_SH, (k % MACRO) * O_SH + O_SH)
                tsm_ = tmppool.tile([128, O_SH], F16, tag="ts", name=f"ts{k}")
                nc.vector.tensor_tensor(tsm_[:], tm_[:, sl], sm_[:, sl],
                                        op=ALU.mult)
                nc.vector.tensor_tensor(wtm_[:, sl], tsm_[:], wlm_[:, sl],
                                        op=ALU.add)
                w_aps[k] = wtm_[:, sl]

            dummies(0, 6)

            # macro 0: LoRA fold + per-i-tile assembly ahead of the warm-up
            # so wt[0..3] is ready the moment the head group can issue.
            tm, sm, wlm, wtm = ts_staged[0]
            for k in range(MACRO):
                lora_fold(k, wlm)
                assemble_it(k, tm, sm, wlm, wtm)

            dummies(6, 8)

            def group_mms(jlo):
                for jt in range(jlo, jlo + MACRO):
                    for g in range(G):
                        nc.tensor.matmul(
                            po_g[g][:], xg[g][:, jt, :], w_aps[jt],
                            start=(jt == 0), stop=(jt == IT - 1),
                        )

            # macro-stepped head: macro m's LoRA fold + assembly is emitted
            # ahead of macro (m-1)'s 20 head-group matmuls, and its t/s DMA
            # one full macro-step earlier, so each ~5us group window hides
            # the next macro's W pipeline with margin.
            n_macros = IT // MACRO
            for m in range(1, n_macros + 1):
                if m < n_macros:
                    k0 = m * MACRO
                    if m % 2 == 1 and m <= 5:
                        stage_x_chunk((m + 1) // 2)
                    if m + 1 < n_macros:
                        ts_staged[m + 1] = stage_ts_macro(k0 + MACRO)
                    tm, sm, wlm, wtm = ts_staged[m]
                    for kk in range(k0, k0 + MACRO):
                        lora_fold(kk, wlm)
                    for kk in range(k0, k0 + MACRO):
                        assemble_it(kk, tm, sm, wlm, wtm)
                    if m >= 6:
                        # prefetch the first steady-state x tiles so the
                        # m-loop starts without a DMA bubble
                        for mt_pre in (G + 2 * (m - 6), G + 2 * (m - 6) + 1):
                            xa_p = xpool.tile([128, IT, 128], BF16, tag="xa",
                                              name=f"xa_pre{mt_pre}")
                            nc.sync.dma_start(xa_p[:],
                                              xt.ap()[:, mt_pre, :, :])
                            xa_pre[mt_pre] = xa_p
                group_mms((m - 1) * MACRO)

            def drain(po, mt):
                ev = opool.tile([128, O_SH], BF16, tag="ev")
                nc.scalar.copy(ev[:], po[:])
                nc.sync.dma_start(out.ap()[mt * 128:(mt + 1) * 128, :], ev[:])

            for g in range(G):
                drain(po_g[g], g)

            # ---- steady-state m-loop ----
            for mt in range(G, MT):
                if mt in xa_pre:
                    xa = xa_pre[mt]
                else:
                    xa = xpool.tile([128, IT, 128], BF16, tag="xa")
                    nc.sync.dma_start(xa[:], xt.ap()[:, mt, :, :])
                po = pp_o.tile([128, O_SH], F32, tag="po")
                for it in range(IT):
                    nc.tensor.matmul(
                        po[:], xa[:, it, :], w_aps[it],
                        start=(it == 0), stop=(it == IT - 1),
                    )
                drain(po, mt)

    nc.compile()
    return nc


_NC_CACHE = {}


def _get_nc():
    if "nc" not in _NC_CACHE:
        _NC_CACHE["nc"] = _build_nc()
    return _NC_CACHE["nc"]


def prepare_in_maps(x, w_codes, w_scales, lora_a, lora_b):
    """Host-side sharding + layout prep (casts/folds/recodes only)."""
    xm = np.ascontiguousarray(x.reshape(M, I))
    # xt[p, mt, t, mm] = x[mt*128+mm, t*128+p], bf16
    xtl = (
        xm.T.reshape(IT, 128, MT, 128)
        .transpose(1, 2, 0, 3)
        .astype(BF16_NP)
    )
    xtl = np.ascontiguousarray(xtl)

    la = np.ascontiguousarray(
        (LORA_SCALE * lora_a.astype(np.float64)).astype(BF16_NP)
    )

    tvals_full = NF4_TABLE[w_codes].astype(np.float16)          # [O, I]
    scales_full = np.repeat(
        w_scales.astype(np.float16), BLK, axis=1
    )                                                           # [O, I]

    in_maps = []
    for c in range(N_CORES):
        o_lo, o_hi = c * O_SH, (c + 1) * O_SH
        tvals_t = np.ascontiguousarray(tvals_full[o_lo:o_hi].T)
        scales_t = np.ascontiguousarray(scales_full[o_lo:o_hi].T)
        lb_t = np.ascontiguousarray(lora_b[o_lo:o_hi].T.astype(BF16_NP))
        in_maps.append(
            {
                "xt": xtl,
                "tvals": tvals_t,
                "scales": scales_t,
                "la": la,
                "lb": lb_t,
            }
        )
    return in_maps


def run(in_maps, trace=False, retries=2):
    nc = _get_nc()
    last = None
    for attempt in range(retries + 1):
        try:
            return run_bass_kernel_spmd(
                nc, in_maps, core_ids=list(range(N_CORES)), trace=trace
            )
        except Exception as e:  # transient NRT/axon device errors
            last = e
            if attempt == retries:
                raise
            import time as _time

            _time.sleep(5)
    raise last


def kernel(x, w_codes, w_scales, lora_a, lora_b):
    in_maps = prepare_in_maps(x, w_codes, w_scales, lora_a, lora_b)
    res = run(in_maps, trace=False)
    out = np.concatenate(
        [np.asarray(res.results[c]["out"]) for c in range(N_CORES)], axis=1
    )
    return out.reshape(B, S, O).astype(np.float32)
